# revision 47
# baseline (speedup 1.0000x reference)
"""Trainium2 Bass kernel for nn_Block_softmoe (dense transformer block, B=4 S=2048 C=256 H=8).

Strategy (v4)
-------------
Sharding: 8 cores = (batch b, query-half). Each core computes the full block for
1024 query rows of one batch. K/V are computed per-core over that batch's keys
(2x redundant K/V projection; tiny at dim 256). No collectives.

Mask compaction: the key mask (Bernoulli 0/1) is applied on the host by
gathering only the kept key rows, so L ~= 1100, KC = L/128 chunks of keys.

Cost-model facts this kernel is built around:
  - matmul cost = out-free-size x cycles_per_row (bf16 / f32r>=256: 1.0); the
    stationary (lhsT) load is free -> stream the SMALL operand.
  - only ACT has exp; DVE fakes it with the Schraudolph bit trick
    (int16(a*y+b) bitcast to bf16; int16 saturation gives -0.0 for masked
    keys). The 8*KC exp tiles are SPLIT across ACT and DVE by a Bresenham
    interleave (DVE_EXP_FRAC), since both engines are equally precious.
  - every HWDGE dma_start serializes ~625ns on the single HWDGE queue ->
    consolidated input DMAs, x tensors split in column halves for an early
    start, bulk loads via the Pool engine's SWDGE path (GPSIMD cannot touch
    PSUM so Pool only gets SBUF/DRAM work: DMAs + Vone ones-memsets).
  - matmul start=True lazily zeroes the whole 2KB psum bank (pending-zero),
    so only the FIRST write into an accumulation bank may set it.
  - ALL of PSUM is one 4-deep ring of [128,1024] f32 tiles (8 banks) shared
    by proj / scores / attnV chains / transposes / MLP: 4-deep decouples the
    PE->exp->PE semaphore round trip, which otherwise rate-limits the phase.

Dataflow per core (SQ=1024 queries, L keys):
  QT = WqT.T @ xqT   [256, SQ] f32r (feature-major)   KT likewise [256, L]
  Vone[kc] [128, 8*33] bf16: per head h cols h*33..h*33+31 = V feats, +1 ones
  scores (h,kc): psum[128, SQ] = KT_h[kc].T @ QT_h   (PE, streams queries);
    one V-projection unit per kc batch (clumps would starve the exp engines)
  P[h,kc] = exp(scale*scores + maskbias) -> bf16     (ACT exp | DVE bit trick)
  attnV (qc: 128-query chunk): po[128, 264] += P[h,kc][:,qc].T @ Vone[kc][h]
    -> ONE 33-wide moving stream yields attn.V AND the softmax denominator.
  normalize: rec = 1/po[:,:,32] (DVE), xout[q,c] = po * rec (DVE bcast AP)
  transpose xout -> feature-major xoutT via PE identity-transpose, written
    into bank 1 of the chain's own ring tile (keeps chain allocations >4
    FIFO slots apart so no chain ever waits another chain's normalize);
    consumers lag the producer chain by 1-2 chains so the in-order PE queue
    never stalls on a fresh dependency.
  MLP in 256-col quarter strips overlapping the attnV tail:
    h1T = gelu(W1T.T @ xoutT + b1) (ACT), pf = h1T.T @ W2T (PE),
    out = pf + xout (residual fused into the DVE output add), paired 256-row
    output DMAs.

Self-contained: hardcodes all shapes; compiled NEFF cached per L.
"""

import os
import sys

for _p in ("/opt/trn_rl_repo", "/root/.axon_site/_ro/trn_rl_repo"):
    if os.path.isdir(_p) and _p not in sys.path:
        sys.path.append(_p)

import ml_dtypes
import numpy as np

import concourse.bacc as bacc
import concourse.tile as tile
from concourse import mybir
from concourse.bass_utils import run_bass_kernel_spmd

B, S, C, H, HD = 4, 2048, 256, 8, 32
NCORES = 8
SQ = 1024                      # query rows per core
NQC = SQ // 128                # query chunks for attnV
SCALE = float(HD) ** -0.5
F32 = mybir.dt.float32
F32R = mybir.dt.float32r
BF16 = mybir.dt.bfloat16
I16 = mybir.dt.int16
AF = mybir.ActivationFunctionType
ALU = mybir.AluOpType
NEG = -1e30

# Schraudolph exp in bf16 bits: int16(A*y + B) viewed as bf16 ~= exp(y).
A_EXP = 2.0 ** 7 / np.log(2.0)
B_EXP = 127.0 * 2.0 ** 7 - 4.7
MB2_MASKED = -1e6              # saturates the int16 -> -32768 -> bf16 -0.0

DVE_EXP_FRAC = float(os.environ.get("K_DVEF", "0.432"))  # exp units on DVE
RING_BUFS = int(os.environ.get("K_RING", "4"))   # scores/proj/mlp psum ring depth
PO_BUFS = int(os.environ.get("K_PO", "0"))       # 0: attnV chains share the ring

_cache: dict = {}


def _build(L: int, use_bv: bool, use_b: bool = True):
    """Build the single-core program (SPMD across 8 cores)."""
    KC = L // 128
    nc = bacc.Bacc("TRN2", target_bir_lowering=False, debug=False, num_devices=NCORES)

    # ---- I/O ----
    d_xqT = nc.dram_tensor("xqT", [C, SQ], BF16, kind="ExternalInput")
    d_xkT = nc.dram_tensor("xkT", [C, L], BF16, kind="ExternalInput")
    d_wqT = nc.dram_tensor("wqT", [C, C], BF16, kind="ExternalInput")
    d_wkT = nc.dram_tensor("wkT", [C, C], BF16, kind="ExternalInput")
    d_wvT = nc.dram_tensor("wvT", [C, C], BF16, kind="ExternalInput")
    d_w1T = nc.dram_tensor("w1T", [C, C], BF16, kind="ExternalInput")
    d_w2TA = nc.dram_tensor("w2TA", [C, C], F32R, kind="ExternalInput")
    d_mbb = nc.dram_tensor("mbb", [128, 2 * KC], F32, kind="ExternalInput")  # mb|mb2
    d_ident = nc.dram_tensor("ident", [128, 128], BF16, kind="ExternalInput")
    d_bqk1 = nc.dram_tensor("bqk1", [128, 6], F32, kind="ExternalInput")  # bq|bk|b1
    d_bvrow = nc.dram_tensor("bvrow", [1, C], F32R, kind="ExternalInput")
    d_out = nc.dram_tensor("out", [SQ, C], F32, kind="ExternalOutput")

    # DVE/ACT exp assignment: Bresenham spread of the DVE units over 8*KC
    nu = 8 * KC
    nd = min(nu, round(nu * DVE_EXP_FRAC))
    dve_units = {u for u in range(nu)
                 if (u * nd) // nu != ((u + 1) * nd) // nu}

    with tile.TileContext(nc) as tc:
        with tc.tile_pool(name="persist", bufs=1) as pp, \
             tc.tile_pool(name="pt", bufs=1) as ptp, \
             tc.tile_pool(name="work", bufs=3) as wp, \
             tc.tile_pool(name="ps_r", bufs=RING_BUFS, space="PSUM") as ps_r:

            # ---- consolidated tiles (chunk-major columns) ----
            xqT = pp.tile([128, 2 * SQ], BF16, tag="xqT", name="xqT")    # cols kk*SQ+q
            xkT = pp.tile([128, 2 * L], BF16, tag="xkT", name="xkT")     # cols kk*L+t
            wqT = pp.tile([128, 2 * C], BF16, tag="wqT", name="wqT")     # cols kk*C+f
            wkT = pp.tile([128, 2 * C], BF16, tag="wkT", name="wkT")
            wvT = pp.tile([128, 2 * C], BF16, tag="wvT", name="wvT")
            w1T = pp.tile([128, 2 * C], BF16, tag="w1T", name="w1T")
            w2TA = pp.tile([128, 2 * C], F32R, tag="w2TA", name="w2TA")  # cols cc*C+f
            mbb = pp.tile([128, 2 * KC], F32, tag="mbb", name="mbb")
            identb = pp.tile([128, 128], BF16, tag="identb", name="identb")

            def _chunks(dram, tile_t, nch, w, dt_):
                # one DMA: DRAM [nch*128, w] -> SBUF [128, nch*w] chunk-major
                nc_ = dram[:, :].rearrange("(c p) w -> p c w", c=nch)
                nc.sync.dma_start(out=tile_t[:, :].rearrange("p (c w) -> p c w", c=nch),
                                  in_=nc_)

            # critical path on HWDGE: weights first (tiny), then x in
            # column halves so proj unit 0 starts before the full load
            _chunks(d_wqT, wqT, 2, C, BF16)
            _chunks(d_wkT, wkT, 2, C, BF16)
            xq_r = xqT[:, :].rearrange("p (c w) -> p c w", c=2)
            xk_r = xkT[:, :].rearrange("p (c w) -> p c w", c=2)
            nc.sync.dma_start(out=xq_r[:, :, 0:512],
                              in_=d_xqT[:, 0:512].rearrange("(c p) w -> p c w", c=2))
            nc.sync.dma_start(out=xk_r[:, :, 0:512],
                              in_=d_xkT[:, 0:512].rearrange("(c p) w -> p c w", c=2))
            nc.sync.dma_start(out=xq_r[:, :, 512:SQ],
                              in_=d_xqT[:, 512:SQ].rearrange("(c p) w -> p c w", c=2))
            nc.sync.dma_start(out=xk_r[:, :, 512:L],
                              in_=d_xkT[:, 512:L].rearrange("(c p) w -> p c w", c=2))
            # bulk loads via Pool SWDGE (off the HWDGE queue)
            nc.gpsimd.dma_start(out=mbb, in_=d_mbb[:, :])
            nc.gpsimd.dma_start(out=wvT[:, :].rearrange("p (c w) -> p c w", c=2),
                                in_=d_wvT[:, :].rearrange("(c p) w -> p c w", c=2))
            if use_b:
                bqk1 = pp.tile([128, 6], F32, tag="bqk1", name="bqk1")
                nc.gpsimd.dma_start(out=bqk1, in_=d_bqk1[:, :])
            nc.gpsimd.dma_start(out=identb, in_=d_ident[:, :])
            nc.gpsimd.dma_start(out=w1T[:, :].rearrange("p (c w) -> p c w", c=2),
                                in_=d_w1T[:, :].rearrange("(c p) w -> p c w", c=2))
            nc.gpsimd.dma_start(out=w2TA[:, :].rearrange("p (c w) -> p c w", c=2),
                                in_=d_w2TA[:, :].rearrange("(c p) w -> p c w", c=2))
            if use_bv:
                bvrow = pp.tile([1, C], F32R, tag="bvrow", name="bvrow")
                onesr = pp.tile([1, 128], F32R, tag="onesr", name="onesr")
                nc.gpsimd.dma_start(out=bvrow, in_=d_bvrow[:, :])
                nc.vector.memset(onesr, 1.0)

            mb = mbb[:, 0:KC]
            mb2 = mbb[:, KC:2 * KC]

            # warm up the ACT exp table at t=0: the auto-inserted
            # LoadActFuncSet lands before this dummy, off the critical path
            warm = wp.tile([128, 1], F32, tag="warm", name="warm")
            warm2 = wp.tile([128, 1], BF16, tag="warm2", name="warm2")
            nc.vector.memset(warm, 0.0)
            nc.scalar.activation(out=warm2, in_=warm, func=AF.Exp)


            # ---- persistent intermediates ----
            QT = [pp.tile([128, SQ], F32R, tag=f"QT{m}", name=f"QT{m}") for m in range(2)]
            KT = [pp.tile([128, L], F32R, tag=f"KT{m}", name=f"KT{m}") for m in range(2)]
            Vone = [pp.tile([128, H * 33], BF16, tag=f"Vone{sc}", name=f"Vone{sc}")
                    for sc in range(KC)]
            # feature-major attn output: cols = cc*SQ + q
            xoutT = pp.tile([128, 2 * SQ], BF16, tag="xoutT", name="xoutT")
            h1T = [pp.tile([128, SQ], F32R, tag=f"h1T{j}", name=f"h1T{j}") for j in range(2)]
            PT = {}

            def _ps():
                # one unified PSUM ring shared by proj / scores / transpose /
                # MLP / attnV chains; users slice what they need
                return ps_r.tile([128, SQ], F32, tag="ring", name="ring")

            kchunks = [(o, min(512, L - o)) for o in range(0, L, 512)]

            def emit_qk_proj(m):
                for n in range(2):  # Q: SQ/512
                    pq = _ps()
                    for kk in range(2):
                        nc.tensor.matmul(out=pq[:, 0:512], lhsT=wqT[:, kk * C + m * 128:kk * C + (m + 1) * 128],
                                         rhs=xqT[:, kk * SQ + n * 512:kk * SQ + (n + 1) * 512],
                                         start=(kk == 0), stop=(kk == 1))
                    if use_b:
                        nc.vector.tensor_scalar_add(out=QT[m][:, n * 512:(n + 1) * 512],
                                                    in0=pq[:, 0:512], scalar1=bqk1[:, m:m + 1])
                    else:
                        nc.scalar.copy(out=QT[m][:, n * 512:(n + 1) * 512], in_=pq[:, 0:512])
                for o, w in kchunks:
                    pk = _ps()
                    for kk in range(2):
                        nc.tensor.matmul(out=pk[:, 0:w], lhsT=wkT[:, kk * C + m * 128:kk * C + (m + 1) * 128],
                                         rhs=xkT[:, kk * L + o:kk * L + o + w],
                                         start=(kk == 0), stop=(kk == 1))
                    if use_b:
                        nc.vector.tensor_scalar_add(out=KT[m][:, o:o + w], in0=pk[:, 0:w],
                                                    scalar1=bqk1[:, 2 + m:3 + m])
                    else:
                        nc.vector.tensor_copy(out=KT[m][:, o:o + w], in_=pk[:, 0:w])

            def emit_v_proj(sc):
                pv = _ps()
                for kk in range(2):
                    nc.tensor.matmul(out=pv[:, :C],
                                     lhsT=xkT[:, kk * L + sc * 128:kk * L + (sc + 1) * 128],
                                     rhs=wvT[:, kk * C:(kk + 1) * C], start=(kk == 0),
                                     stop=(kk == 1) and not use_bv)
                if use_bv:
                    nc.tensor.matmul(out=pv[:, :C], lhsT=onesr[0:1, :],
                                     rhs=bvrow[0:1, :], start=False, stop=True)
                vr = Vone[sc][:, :].rearrange("p (h w) -> p h w", h=H)
                nc.vector.tensor_copy(out=vr[:, :, 0:32],
                                      in_=pv[:, :C].rearrange("p (h w) -> p h w", h=H))
                nc.gpsimd.memset(vr[:, :, 32:33], 1.0)

            uidx = [0]

            def emit_score_exp(h, kc):
                g, j = h // 4, h % 4
                pss = _ps()
                for qn in range(2):
                    nc.tensor.matmul(
                        out=pss[:, qn * 512:(qn + 1) * 512],
                        lhsT=KT[g][32 * j:32 * j + 32, kc * 128:(kc + 1) * 128],
                        rhs=QT[g][32 * j:32 * j + 32, qn * 512:(qn + 1) * 512],
                        start=True, stop=True,
                        tile_position=(32 * j, 0))
                pt_t = ptp.tile([128, SQ], BF16, tag="pt", bufs=8 * KC,
                                name=f"pt{h}_{kc}")
                if uidx[0] in dve_units:
                    nc.vector.tensor_scalar(out=pt_t.bitcast(I16), in0=pss,
                                            scalar1=float(SCALE * A_EXP),
                                            scalar2=mb2[:, kc:kc + 1],
                                            op0=ALU.mult, op1=ALU.add)
                else:
                    nc.scalar.activation(out=pt_t, in_=pss, func=AF.Exp,
                                         bias=mb[:, kc:kc + 1], scale=SCALE)
                uidx[0] += 1
                PT[h, kc] = pt_t

            po_of = {}

            def emit_attn(qc, kc):
                if kc == 0:
                    po_of[qc] = (_ps() if PO_BUFS == 0 else
                                 ps_r.tile([128, H * 33], F32, tag="po",
                                           bufs=PO_BUFS, name="po"))
                po = po_of[qc]
                for h in range(H):
                    # start=True pending-zeroes the WHOLE psum bank, so only
                    # the very first write into the bank may set it.
                    nc.tensor.matmul(
                        out=po[:, h * 33:(h + 1) * 33],
                        lhsT=PT[h, kc][:, qc * 128:(qc + 1) * 128],
                        rhs=Vone[kc][:, h * 33:(h + 1) * 33],
                        start=(kc == 0 and h == 0), stop=(kc == KC - 1),
                        skip_group_check=(h > 0))

            xo_of = {}

            def emit_norm(qc):
                po = po_of[qc][:, 0:H * 33].rearrange("p (h w) -> p h w", h=H)
                rec = wp.tile([128, H, 1], F32, tag="rec", name="rec")
                nc.vector.reciprocal(out=rec, in_=po[:, :, 32:33])
                xo = wp.tile([128, C], BF16, tag="xo", bufs=5, name="xo")
                nc.vector.tensor_mul(out=xo[:, :].rearrange("p (h w) -> p h w", h=H),
                                     in0=po[:, :, 0:32],
                                     in1=rec[:, :, :].broadcast_to((128, H, 32)))
                xo_of[qc] = xo

            def emit_transpose_evac(qc):
                # transpose into bank 1 of the chain's own ring tile (cols
                # 512:768, untouched by the [0:264] attnV accumulation) so no
                # extra ring slot is consumed -> consecutive chains stay >4
                # FIFO slots apart and never wait on a chain's normalize
                xo = xo_of[qc]
                ptile = po_of.pop(qc)
                for cc in range(2):
                    nc.tensor.transpose(out=ptile[:, 512 + cc * 64:512 + (cc + 1) * 64].bitcast(BF16),
                                        in_=xo[:, cc * 128:(cc + 1) * 128],
                                        identity=identb)
                xoT = xoutT[:, :].rearrange("p (c q) -> p c q", c=2)
                nc.vector.tensor_copy(
                    out=xoT[:, :, qc * 128:(qc + 1) * 128],
                    in_=ptile[:, 512:512 + C // 2].bitcast(BF16).rearrange("p (c q) -> p c q", c=2))

            def emit_mlp_h1(n):
                # 256-col strip: queries n*256..(n+1)*256 (query chunks 2n, 2n+1)
                for j in range(2):
                    ph = _ps()
                    for cc in range(2):
                        nc.tensor.matmul(out=ph[:, 0:256], lhsT=w1T[:, cc * C + j * 128:cc * C + (j + 1) * 128],
                                         rhs=xoutT[:, cc * SQ + n * 256:cc * SQ + (n + 1) * 256],
                                         start=(cc == 0), stop=(cc == 1))
                    nc.scalar.activation(out=h1T[j][:, n * 256:(n + 1) * 256], in_=ph[:, 0:256],
                                         func=AF.Gelu,
                                         bias=(bqk1[:, 4 + j:5 + j] if use_b else 0.0))

            def emit_mlp_final(n):
                # residual fused on DVE: out = h1 @ W2T + xout (token-major)
                ot = wp.tile([128, 2 * C], F32, tag="ot", bufs=2, name="ot")
                for si, sc in enumerate(range(2 * n, 2 * n + 2)):
                    pf = _ps()
                    for cc in range(2):
                        nc.tensor.matmul(out=pf[:, :C],
                                         lhsT=h1T[cc][:, sc * 128:(sc + 1) * 128],
                                         rhs=w2TA[:, cc * C:(cc + 1) * C],
                                         start=(cc == 0), stop=(cc == 1))
                    nc.vector.tensor_add(out=ot[:, si * C:(si + 1) * C],
                                         in0=pf[:, :C], in1=xo_of.pop(sc))
                nc.sync.dma_start(
                    out=d_out[2 * n * 128:(2 * n + 2) * 128, :].rearrange("(s p) w -> p s w", s=2),
                    in_=ot[:, :].rearrange("p (s w) -> p s w", s=2))

            # ---- emission schedule ----
            emit_qk_proj(0)
            for h in range(4):
                emit_score_exp(h, 0)
            emit_qk_proj(1)
            for h in range(4, 8):
                emit_score_exp(h, 0)
            emit_v_proj(0)
            # one V-proj unit per kc batch: spreading them avoids clumps of
            # ring-slot waits that would starve the exp engines
            for kc in range(1, KC):
                for h in range(H):
                    emit_score_exp(h, kc)
                emit_v_proj(kc if kc < KC else 0)
            # tail: attnV chains through the same psum ring, with consumers
            # lagged so nothing at the head of the in-order PE queue waits:
            # norm(qc-1) on DVE, transpose+evac(qc-2) on PE (mult long done),
            # MLP h1 for quarter n once evac(2n+1) is emitted, finals one
            # chain later (gelus done by then)
            for qc in range(NQC):
                for kc in range(KC):
                    emit_attn(qc, kc)
                if qc >= 1:
                    emit_norm(qc - 1)
                if qc >= 2:
                    emit_transpose_evac(qc - 2)
                if qc >= 3 and qc % 2 == 1:
                    emit_mlp_h1((qc - 3) // 2)
                if qc >= 4 and qc % 2 == 0:
                    emit_mlp_final((qc - 4) // 2)
            def emit_mlp_h1_eighth(e):
                # 128-col h1 strip for query chunk e (tail-end latency trim)
                for j in range(2):
                    ph = _ps()
                    for cc in range(2):
                        nc.tensor.matmul(out=ph[:, 0:128],
                                         lhsT=w1T[:, cc * C + j * 128:cc * C + (j + 1) * 128],
                                         rhs=xoutT[:, cc * SQ + e * 128:cc * SQ + (e + 1) * 128],
                                         start=(cc == 0), stop=(cc == 1))
                    nc.scalar.activation(out=h1T[j][:, e * 128:(e + 1) * 128], in_=ph[:, 0:128],
                                         func=AF.Gelu,
                                         bias=(bqk1[:, 4 + j:5 + j] if use_b else 0.0))

            def emit_mlp_final_single(sc):
                pf = _ps()
                for cc in range(2):
                    nc.tensor.matmul(out=pf[:, :C],
                                     lhsT=h1T[cc][:, sc * 128:(sc + 1) * 128],
                                     rhs=w2TA[:, cc * C:(cc + 1) * C],
                                     start=(cc == 0), stop=(cc == 1))
                ots = wp.tile([128, C], F32, tag="ots", bufs=2, name="ots")
                nc.vector.tensor_add(out=ots, in0=pf[:, :C], in1=xo_of.pop(sc))
                nc.sync.dma_start(out=d_out[sc * 128:(sc + 1) * 128, :], in_=ots)

            emit_norm(NQC - 1)
            emit_transpose_evac(NQC - 2)
            emit_mlp_h1_eighth(NQC - 2)
            emit_mlp_final(NQC // 2 - 2)
            emit_transpose_evac(NQC - 1)
            emit_mlp_final_single(NQC - 2)
            emit_mlp_h1_eighth(NQC - 1)
            emit_mlp_final_single(NQC - 1)

    nc.compile()
    return nc


def _prep_inputs(x, mask, Wq, bq, Wk, bk, Wv, bv, W1, b1, W2, b2):
    """Host-side sharding + layout prep. Returns (L, in_maps, use_bv, use_b)."""
    x = np.ascontiguousarray(x, dtype=np.float32)
    keeps = [np.flatnonzero(mask[b, :S] != 0) for b in range(B)]
    cnts = [len(k) for k in keeps]
    L = max(128, -(-max(cnts) // 128) * 128)
    KC = L // 128

    BF = ml_dtypes.bfloat16
    wqT = np.ascontiguousarray(np.asarray(Wq, np.float32).T.astype(BF))
    wkT = np.ascontiguousarray(np.asarray(Wk, np.float32).T.astype(BF))
    wvT = np.ascontiguousarray(np.asarray(Wv, np.float32).T.astype(BF))
    w1T = np.ascontiguousarray(np.asarray(W1, np.float32).T.astype(BF))
    w2TA = np.ascontiguousarray(W2.T, dtype=np.float32)
    ident = np.eye(128, dtype=ml_dtypes.bfloat16)
    bqk1 = np.stack([
        bq[0:128], bq[128:256], bk[0:128], bk[128:256], b1[0:128], b1[128:256],
    ], axis=1).astype(np.float32)
    bvrow = np.asarray(bv, np.float32).reshape(1, C)
    use_bv = bool(np.any(bv != 0))
    use_b = bool(np.any(bq != 0) or np.any(bk != 0) or np.any(b1 != 0))

    in_maps = []
    for core in range(NCORES):
        b, half = core // 2, core % 2
        xb = x[b]                                   # [S, C]
        xqT = np.ascontiguousarray(xb[half * SQ:(half + 1) * SQ].T.astype(BF))  # [C, SQ]
        xk = np.zeros((L, C), dtype=np.float32)
        xk[:cnts[b]] = xb[keeps[b]]
        xkT = np.ascontiguousarray(xk.T.astype(BF))  # [C, L]
        mb1d = np.full(L, NEG, dtype=np.float32)
        mb1d[:cnts[b]] = 0.0
        mb = mb1d.reshape(KC, 128).T                 # [128, KC]
        mb2_1d = np.full(L, MB2_MASKED, dtype=np.float32)
        mb2_1d[:cnts[b]] = B_EXP
        mb2 = mb2_1d.reshape(KC, 128).T              # [128, KC]
        mbb = np.ascontiguousarray(np.concatenate([mb, mb2], axis=1))  # [128, 2KC]
        in_maps.append({
            "xqT": xqT, "xkT": xkT, "wqT": wqT, "wkT": wkT, "wvT": wvT,
            "w1T": w1T, "w2TA": w2TA, "mbb": mbb, "ident": ident,
            "bqk1": bqk1, "bvrow": bvrow,
        })
    return L, in_maps, use_bv, use_b


def kernel(x, mask, Wq, bq, Wk, bk, Wv, bv, W1, b1, W2, b2):
    L, in_maps, use_bv, use_b = _prep_inputs(x, mask, Wq, bq, Wk, bk, Wv, bv, W1, b1, W2, b2)
    key = (L, use_bv, use_b)
    if key not in _cache:
        _cache[key] = _build(L, use_bv, use_b)
    nc = _cache[key]
    res = None
    last_exc = None
    for attempt in range(4):
        try:
            res = run_bass_kernel_spmd(nc, in_maps, core_ids=list(range(NCORES)),
                                       trace=False)
            break
        except Exception as e:  # transient device errors on first exec of a NEFF
            last_exc = e
            import time as _time
            import jax as _jax
            _time.sleep(2.0)
            try:
                _jax.clear_caches()
            except Exception:
                pass
    if res is None:
        raise last_exc
    out = np.empty((B, S, C), dtype=np.float32)
    for core in range(NCORES):
        b, half = core // 2, core % 2
        out[b, half * SQ:(half + 1) * SQ] = res.results[core]["out"]
    if np.any(b2 != 0):
        out += np.asarray(b2, dtype=np.float32)[None, None, :]
    # stash for test harness reuse (timing reruns)
    kernel.last = {"nc": nc, "in_maps": in_maps, "L": L}
    return out
